# revision 9
# baseline (speedup 1.0000x reference)
"""Trainium2 Bass kernel: ConvNeXt MLP + parallel MoE-LoRA (data-parallel over tokens).

Math per token t (D=512, Dh=2048, E=3 experts, r=8, top-k=2):
    base = gelu(x @ W1 + b1) @ W2 + b2
    g_e  = gelu(x @ w_down[e]) * wts[e, t]          (wts from top-k routing)
    out  = base + sum_e g_e @ w_up[e]

Strategy (per NeuronCore, 8 cores data-parallel on the token dim):
  - all matmul operands pre-cast to bf16 on host (x, W1, W2, wd, wu, ident)
    so the device does zero weight casting and x DMA is halved.
  - tokens tiled 128 at a time; supergroups (SG) of 4 tiles (512 tokens).
  - x tile [128t, 512d] PE-transposed into xT [128d, t] (bf16 direct).
  - MM1: hT[h,t] = W1_chunk.T @ xT, fused bias+gelu on ScalarE -> actT bf16.
    W1 SBUF layout is h-major so MM1 h=0 can start after 1/4 of the W1 DMA.
  - MM2: out[t,d] accumulates the rank-24 LoRA-up matmul FIRST, then the
    16 h-chunks, in one PSUM group; b2 added during the drain (DVE) from a
    host-replicated [128,512] bias tile.
  - LoRA chain for SG n+1 (down-matmuls -> gelu -> routing scale -> PE
    transpose -> SBUF) is emitted during SG n so its ScalarE/DVE stages
    hide under MM1/MM2 and the PE never waits on it.
  - routing weights wts[e,t] computed on device in a DVE prologue.
  - batched DMAs: x in per-SG [512 tok], out per 2 tiles, W1 in 4 / W2 in
    2 chunks.
"""

import os
import numpy as np

P = 128
D = 512
DH = 2048
E = 3
R = 8
ER = E * R  # 24
NH = DH // P  # 16
NDC = D // P  # 4
N_CORES = 8
T_FULL = 64 * 28 * 28  # 50176
TC = T_FULL // N_CORES  # 6272
GROUP_TILES = 4

_CACHE = {}


def _build(tc_tokens):
    import concourse.bacc as bacc
    import concourse.tile as tile
    import concourse.mybir as mybir
    from contextlib import ExitStack

    f32 = mybir.dt.float32
    bf16 = mybir.dt.bfloat16
    i32 = mybir.dt.int32
    AF = mybir.ActivationFunctionType
    OP = mybir.AluOpType

    nt = tc_tokens // P  # token tiles (49)
    assert tc_tokens % P == 0
    ngroups = (nt + GROUP_TILES - 1) // GROUP_TILES

    nc = bacc.Bacc("TRN2", target_bir_lowering=False, debug=False,
                   num_devices=N_CORES)

    xt = nc.dram_tensor("x", [D, tc_tokens], bf16, kind="ExternalInput").ap()
    w1 = nc.dram_tensor("w1", [P, NH * D], bf16, kind="ExternalInput").ap()
    w2 = nc.dram_tensor("w2", [P, NH * D], bf16, kind="ExternalInput").ap()
    b1 = nc.dram_tensor("b1", [DH], f32, kind="ExternalInput").ap()
    b2r = nc.dram_tensor("b2r", [P, D], f32, kind="ExternalInput").ap()
    wd = nc.dram_tensor("wd", [D, ER], bf16, kind="ExternalInput").ap()
    wu = nc.dram_tensor("wu", [ER, D], bf16, kind="ExternalInput").ap()
    tkp = nc.dram_tensor("tkp", [tc_tokens, 2], f32, kind="ExternalInput").ap()
    tki = nc.dram_tensor("tki", [tc_tokens, 4], i32, kind="ExternalInput").ap()
    ident_d = nc.dram_tensor("ident", [P, P], bf16, kind="ExternalInput").ap()
    out = nc.dram_tensor("out", [tc_tokens, D], f32, kind="ExternalOutput").ap()

    def grp(n):
        ng = min(GROUP_TILES, nt - n * GROUP_TILES)
        return ng, ng * P, n * GROUP_TILES

    with tile.TileContext(nc) as tc, ExitStack() as ctx:
        cons = ctx.enter_context(tc.tile_pool(name="cons", bufs=1))
        xin = ctx.enter_context(tc.tile_pool(name="xin", bufs=4))
        actp = ctx.enter_context(tc.tile_pool(name="actp", bufs=2))
        outp = ctx.enter_context(tc.tile_pool(name="outp", bufs=2))
        gp = ctx.enter_context(tc.tile_pool(name="gp", bufs=3))
        ps_h = ctx.enter_context(tc.tile_pool(name="ps_h", bufs=3, space="PSUM"))
        ps_o = ctx.enter_context(tc.tile_pool(name="ps_o", bufs=3, space="PSUM"))
        ps_lg = ctx.enter_context(tc.tile_pool(name="ps_lg", bufs=1, space="PSUM"))
        ps_gt = ctx.enter_context(tc.tile_pool(name="ps_gt", bufs=1, space="PSUM"))

        # ---- identity (needed by the very first transposes) ----
        ident_b = cons.tile([P, P], bf16)
        nc.sync.dma_start(ident_b[:], ident_d)

        # ---- x arrives host-transposed: DMA xT [128d(c), (c, t)] per SG ----
        xT = {}

        def dma_xT(n):
            ng, G, t0 = grp(n)
            tok0 = t0 * P
            t = xin.tile([P, NDC * G], bf16, tag="xT", name=f"xT{n}")
            nc.sync.dma_start(
                t.rearrange("p (c t) -> p c t", c=NDC),
                xt.rearrange("(c p) t -> p c t", p=P)[:, :, tok0:tok0 + G])
            xT[n] = t

        dma_xT(0)
        dma_xT(1)

        # ---- small/routing inputs first: the LoRA bootstrap + routing DVE
        # prologue only needs ~0.7 MB before the big weights stream in ----
        tp_sb = cons.tile([P, nt * 2], f32)
        nc.sync.dma_start(tp_sb.rearrange("p (n k) -> p n k", k=2),
                          tkp.rearrange("(n p) k -> p n k", p=P))
        ti_sb = cons.tile([P, nt * 4], i32)
        nc.sync.dma_start(ti_sb.rearrange("p (n k) -> p n k", k=4),
                          tki.rearrange("(n p) k -> p n k", p=P))
        wdr = cons.tile([P, NDC * ER], bf16)
        nc.scalar.dma_start(wdr.rearrange("p (c e) -> p c e", c=NDC),
                            wd.rearrange("(c p) e -> p c e", p=P))
        wur = cons.tile([ER, D], bf16)
        nc.scalar.dma_start(wur[:], wu)
        b1_sb = cons.tile([P, NH], f32)
        nc.scalar.dma_start(b1_sb[:], b1.rearrange("(c p) -> p c", p=P))

        # ---- W1: host-prepermuted, 4 chunk-tiles so MM1 h=0 only waits on
        # the first 0.5 MB ----
        # w1 dram layout: w1[p, h*512 + c*128 + q] = W1[c*128 + p, h*128 + q]
        W1c = []
        for hh in range(4):
            t = cons.tile([P, 4 * D], bf16, tag=f"w1c{hh}", name=f"w1c{hh}")
            nc.scalar.dma_start(t[:], w1[:, hh * 2048:(hh + 1) * 2048])
            W1c.append(t)

        if 2 < ngroups:
            dma_xT(2)

        # ---- W2: host-prepermuted [128, (h d)], 2 chunk-tiles ----
        # w2 dram layout: w2[p, h*512 + d] = W2[h*128 + p, d]
        W2c = []
        for k in range(2):
            t = cons.tile([P, 8 * D], bf16, tag=f"w2c{k}", name=f"w2c{k}")
            nc.gpsimd.dma_start(t[:], w2[:, k * 4096:(k + 1) * 4096])
            W2c.append(t)
        b2rep = cons.tile([P, D], f32)
        nc.gpsimd.dma_start(b2rep[:], b2r)

        # ---- routing weights prologue: wts[p, e*nt + n] ----
        idxf = cons.tile([P, nt * 2], f32)
        iv = ti_sb.rearrange("p (n k two) -> p n k two", k=2, two=2)
        nc.vector.tensor_copy(
            idxf.rearrange("p (n k one) -> p n k one", k=2, one=1),
            iv[:, :, :, 0:1])
        wts = cons.tile([P, E * nt], f32)
        for e in range(E):
            eq = cons.tile([P, nt * 2], f32, tag="eq", name=f"eq{e}")
            nc.vector.tensor_scalar(eq[:], idxf[:], float(e), None,
                                    op0=OP.is_equal)
            nc.vector.tensor_tensor(eq[:], eq[:], tp_sb[:], op=OP.mult)
            nc.vector.tensor_reduce(wts[:, e * nt:(e + 1) * nt],
                                    eq.rearrange("p (n k) -> p n k", k=2),
                                    axis=mybir.AxisListType.X, op=OP.add)

        # ---- pipeline stage emitters ----
        gts = {}

        def emit_lora_down(n):
            # pg_all[t, (i, er)] for the whole SG in one PSUM bank, then
            # gelu (ScalarE) and routing scale (DVE) -> g2 bf16
            ng, G, t0 = grp(n)
            pg = ps_lg.tile([P, ng * ER], f32, tag="lg", name=f"pg{n}")
            xTt = xT[n]
            for i in range(ng):
                for c in range(NDC):
                    nc.tensor.matmul(
                        pg[:, i * ER:(i + 1) * ER],
                        xTt[:, c * G + i * P: c * G + (i + 1) * P],
                        wdr[:, c * ER:(c + 1) * ER],
                        start=(c == 0), stop=(c == NDC - 1))
            g_sb = gp.tile([P, ng * ER], f32, tag="g", name=f"g{n}")
            nc.scalar.activation(g_sb[:], pg[:], AF.Gelu)
            g2 = gp.tile([P, ng * ER], bf16, tag="g2", name=f"g2{n}")
            for i in range(ng):
                tt = t0 + i
                for e in range(E):
                    nc.vector.tensor_scalar(
                        g2[:, i * ER + e * R: i * ER + (e + 1) * R],
                        g_sb[:, i * ER + e * R: i * ER + (e + 1) * R],
                        wts[:, e * nt + tt: e * nt + tt + 1], None, op0=OP.mult)
            return g2

        def emit_g_xpose(n, g2):
            ng, G, t0 = grp(n)
            pgt = ps_gt.tile([ER, G], bf16, tag="lgt", name=f"pgt{n}")
            for i in range(ng):
                nc.tensor.matmul(pgt[:, i * P:(i + 1) * P],
                                 g2[:, i * ER:(i + 1) * ER],
                                 ident_b[:], is_transpose=True,
                                 start=True, stop=True)
            gt = gp.tile([ER, G], bf16, tag="gt", name=f"gt{n}")
            nc.vector.tensor_copy(gt[:], pgt[:])
            gts[n] = gt

        # ---- bootstrap: LoRA down-matmuls for SG 0 and 1 (pure PE work);
        # their g-transposes wait on DVE scaling, so they are emitted after
        # MM1(0) inside the loop to keep the in-order PE stream flowing ----
        g2_boot = [emit_lora_down(0)]
        if 1 < ngroups:
            g2_boot.append(emit_lora_down(1))

        # ---- main loop over supergroups ----
        for n in range(ngroups):
            ng, G, t0 = grp(n)
            if n + 3 < ngroups:
                dma_xT(n + 3)

            # MM1 + bias + gelu -> actT [128h, (h_chunk, t)] bf16
            actT = actp.tile([P, NH * G], bf16, tag="actT", name=f"actT{n}")
            xTt = xT[n]
            for h in range(NH):
                ph = ps_h.tile([P, D], f32, tag="ph", name=f"ph{n}_{h}")
                for c in range(NDC):
                    nc.tensor.matmul(
                        ph[:, :G],
                        W1c[h // 4][:, (h % 4) * D + c * P: (h % 4) * D + (c + 1) * P],
                        xTt[:, c * G:(c + 1) * G],
                        start=(c == 0), stop=(c == NDC - 1))
                nc.scalar.activation(actT[:, h * G:(h + 1) * G], ph[:, :G],
                                     AF.Gelu, bias=b1_sb[:, h:h + 1], scale=1.0)

            if n == 0:
                for k, g2b in enumerate(g2_boot):
                    emit_g_xpose(k, g2b)

            # stage SG n+2 LoRA down (PE work now; its ScalarE/DVE stages
            # drain during MM2 below)
            g2_next = None
            if n + 2 < ngroups:
                g2_next = emit_lora_down(n + 2)

            # MM2 per token tile: LoRA-up first, then 16 h-chunks
            o4 = outp.tile([P, ng * D], f32, tag="o4", name=f"o4_{n}")
            gt = gts[n]
            for i in range(ng):
                tt = t0 + i
                po = ps_o.tile([P, D], f32, tag="po", name=f"po{tt}")
                nc.tensor.matmul(po[:], gt[:, i * P:(i + 1) * P], wur[:],
                                 start=True, stop=False)
                for h in range(NH):
                    nc.tensor.matmul(
                        po[:],
                        actT[:, h * G + i * P: h * G + (i + 1) * P],
                        W2c[h // 8][:, (h % 8) * D:(h % 8 + 1) * D],
                        start=False, stop=(h == NH - 1))
                nc.vector.tensor_tensor(o4[:, i * D:(i + 1) * D], po[:],
                                        b2rep[:], op=OP.add)
                if i % 2 == 1 or i == ng - 1:
                    lo = (i // 2) * 2
                    cnt = i - lo + 1
                    nc.gpsimd.dma_start(
                        out[(t0 + lo) * P: (t0 + lo + cnt) * P, :].rearrange(
                            "(i p) d -> p i d", p=P),
                        o4[:, lo * D:(lo + cnt) * D].rearrange(
                            "p (i d) -> p i d", i=cnt))

            del xT[n]
            del gts[n]
            if g2_next is not None:
                emit_g_xpose(n + 2, g2_next)

    nc.compile()
    return nc


def _get_nc():
    key = ("full", TC)
    if key not in _CACHE:
        _CACHE[key] = _build(TC)
    return _CACHE[key]


def _make_in_maps(inputs, tc_tokens=TC, n_cores=N_CORES):
    import ml_dtypes
    bf16 = ml_dtypes.bfloat16

    x = np.asarray(inputs["x"], dtype=np.float32)
    T = x.size // D
    x_flat = x.reshape(T, D).astype(bf16)
    # pre-permute to the SBUF layouts (see _build) for contiguous DMA
    W1 = np.ascontiguousarray(
        np.asarray(inputs["W1"], dtype=np.float32).astype(bf16)
        .reshape(NDC, P, NH, P).transpose(1, 2, 0, 3).reshape(P, NH * D))
    W2 = np.ascontiguousarray(
        np.asarray(inputs["W2"], dtype=np.float32).astype(bf16)
        .reshape(NH, P, D).transpose(1, 0, 2).reshape(P, NH * D))
    b1 = np.ascontiguousarray(inputs["b1"], dtype=np.float32)
    b2 = np.asarray(inputs["b2"], dtype=np.float32)
    b2rep = np.ascontiguousarray(np.broadcast_to(b2[None, :], (P, D)))
    wdn = np.ascontiguousarray(
        np.asarray(inputs["w_down"], dtype=np.float32)
        .transpose(1, 0, 2).reshape(D, ER).astype(bf16))
    wup = np.ascontiguousarray(
        np.asarray(inputs["w_up"], dtype=np.float32).reshape(ER, D).astype(bf16))
    tkp = np.ascontiguousarray(inputs["topk_probs"], dtype=np.float32)
    tki_in = np.asarray(inputs["topk_indices"])
    tki = np.zeros((T, 4), dtype=np.int32)
    tki[:, 0] = tki_in[:, 0]
    tki[:, 2] = tki_in[:, 1]
    ident = np.eye(P, dtype=np.float32).astype(bf16)

    in_maps = []
    for c in range(n_cores):
        sl = slice(c * tc_tokens, (c + 1) * tc_tokens)
        in_maps.append(dict(
            x=np.ascontiguousarray(x_flat[sl].T), w1=W1, w2=W2, b1=b1,
            b2r=b2rep, wd=wdn, wu=wup, tkp=np.ascontiguousarray(tkp[sl]),
            tki=np.ascontiguousarray(tki[sl]), ident=ident))
    return in_maps


def _ensure_ntff_hook():
    """Register the axon NTFF profile hook if the image's antenv lacks it."""
    import sys
    import types
    try:
        from antenv.axon_hooks import get_axon_ntff_profile_hook  # noqa: F401
        return True
    except ImportError:
        pass
    try:
        from trn_agent_boot.trn_boot import _ntff_profile_via_ctypes
        mod = types.ModuleType("antenv.axon_hooks")
        _hook = [None]
        mod.set_axon_ntff_profile_hook = lambda h: _hook.__setitem__(0, h)
        mod.get_axon_ntff_profile_hook = lambda: _hook[0]
        sys.modules["antenv.axon_hooks"] = mod
        import antenv
        antenv.axon_hooks = mod
        mod.set_axon_ntff_profile_hook(
            _ntff_profile_via_ctypes("/opt/axon/libaxon_pjrt.so"))
        return True
    except Exception:
        return False


def kernel(**inputs):
    from concourse.bass_utils import run_bass_kernel_spmd

    nc = _get_nc()
    in_maps = _make_in_maps(inputs)
    trace = bool(int(os.environ.get("KERNEL_TRACE", "0")))
    if trace and not _ensure_ntff_hook():
        trace = False
    res = run_bass_kernel_spmd(nc, in_maps, list(range(N_CORES)), trace=trace)
    if trace:
        _CACHE["last_result"] = res
    out = np.concatenate([res.results[i]["out"] for i in range(N_CORES)], axis=0)
    return out.reshape(np.asarray(inputs["x"]).shape).astype(np.float32)


# revision 10
# speedup vs baseline: 1.0059x; 1.0059x over previous
"""Trainium2 Bass kernel: ConvNeXt MLP + parallel MoE-LoRA (data-parallel over tokens).

Math per token t (D=512, Dh=2048, E=3 experts, r=8, top-k=2):
    base = gelu(x @ W1 + b1) @ W2 + b2
    g_e  = gelu(x @ w_down[e]) * wts[e, t]          (wts from top-k routing)
    out  = base + sum_e g_e @ w_up[e]

Strategy (per NeuronCore, 8 cores data-parallel on the token dim):
  - all matmul operands pre-cast to bf16 on host (x, W1, W2, wd, wu, ident)
    so the device does zero weight casting and x DMA is halved.
  - tokens tiled 128 at a time; supergroups (SG) of 4 tiles (512 tokens).
  - x tile [128t, 512d] PE-transposed into xT [128d, t] (bf16 direct).
  - MM1: hT[h,t] = W1_chunk.T @ xT, fused bias+gelu on ScalarE -> actT bf16.
    W1 SBUF layout is h-major so MM1 h=0 can start after 1/4 of the W1 DMA.
  - MM2: out[t,d] accumulates the rank-24 LoRA-up matmul FIRST, then the
    16 h-chunks, in one PSUM group; b2 added during the drain (DVE) from a
    host-replicated [128,512] bias tile.
  - LoRA chain for SG n+1 (down-matmuls -> gelu -> routing scale -> PE
    transpose -> SBUF) is emitted during SG n so its ScalarE/DVE stages
    hide under MM1/MM2 and the PE never waits on it.
  - routing weights wts[e,t] computed on device in a DVE prologue.
  - batched DMAs: x in per-SG [512 tok], out per 2 tiles, W1 in 4 / W2 in
    2 chunks.
"""

import os
import numpy as np

P = 128
D = 512
DH = 2048
E = 3
R = 8
ER = E * R  # 24
NH = DH // P  # 16
NDC = D // P  # 4
N_CORES = 8
T_FULL = 64 * 28 * 28  # 50176
TC = T_FULL // N_CORES  # 6272
GROUP_TILES = 4

_CACHE = {}


def _build(tc_tokens):
    import concourse.bacc as bacc
    import concourse.tile as tile
    import concourse.mybir as mybir
    from contextlib import ExitStack

    f32 = mybir.dt.float32
    bf16 = mybir.dt.bfloat16
    i32 = mybir.dt.int32
    AF = mybir.ActivationFunctionType
    OP = mybir.AluOpType

    nt = tc_tokens // P  # token tiles (49)
    assert tc_tokens % P == 0
    ngroups = (nt + GROUP_TILES - 1) // GROUP_TILES

    nc = bacc.Bacc("TRN2", target_bir_lowering=False, debug=False,
                   num_devices=N_CORES)

    xt = nc.dram_tensor("x", [D, tc_tokens], bf16, kind="ExternalInput").ap()
    w1 = nc.dram_tensor("w1", [P, NH * D], bf16, kind="ExternalInput").ap()
    w2 = nc.dram_tensor("w2", [P, NH * D], bf16, kind="ExternalInput").ap()
    b1 = nc.dram_tensor("b1", [DH], f32, kind="ExternalInput").ap()
    b2r = nc.dram_tensor("b2r", [P, D], f32, kind="ExternalInput").ap()
    wd = nc.dram_tensor("wd", [D, ER], bf16, kind="ExternalInput").ap()
    wu = nc.dram_tensor("wu", [ER, D], bf16, kind="ExternalInput").ap()
    tkp = nc.dram_tensor("tkp", [tc_tokens, 2], f32, kind="ExternalInput").ap()
    tki = nc.dram_tensor("tki", [tc_tokens, 4], i32, kind="ExternalInput").ap()
    ident_d = nc.dram_tensor("ident", [P, P], bf16, kind="ExternalInput").ap()
    out = nc.dram_tensor("out", [tc_tokens, D], f32, kind="ExternalOutput").ap()

    def grp(n):
        ng = min(GROUP_TILES, nt - n * GROUP_TILES)
        return ng, ng * P, n * GROUP_TILES

    with tile.TileContext(nc) as tc, ExitStack() as ctx:
        cons = ctx.enter_context(tc.tile_pool(name="cons", bufs=1))
        xin = ctx.enter_context(tc.tile_pool(name="xin", bufs=4))
        actp = ctx.enter_context(tc.tile_pool(name="actp", bufs=2))
        outp = ctx.enter_context(tc.tile_pool(name="outp", bufs=2))
        gp = ctx.enter_context(tc.tile_pool(name="gp", bufs=3))
        ps_h = ctx.enter_context(tc.tile_pool(name="ps_h", bufs=3, space="PSUM"))
        ps_o = ctx.enter_context(tc.tile_pool(name="ps_o", bufs=3, space="PSUM"))
        ps_lg = ctx.enter_context(tc.tile_pool(name="ps_lg", bufs=1, space="PSUM"))
        ps_gt = ctx.enter_context(tc.tile_pool(name="ps_gt", bufs=1, space="PSUM"))

        # ---- identity (needed by the very first transposes) ----
        ident_b = cons.tile([P, P], bf16)
        nc.sync.dma_start(ident_b[:], ident_d)

        # ---- x arrives host-transposed: DMA xT [128d(c), (c, t)] per SG ----
        xT = {}

        def dma_xT(n):
            ng, G, t0 = grp(n)
            tok0 = t0 * P
            t = xin.tile([P, NDC * G], bf16, tag="xT", name=f"xT{n}")
            nc.sync.dma_start(
                t.rearrange("p (c t) -> p c t", c=NDC),
                xt.rearrange("(c p) t -> p c t", p=P)[:, :, tok0:tok0 + G])
            xT[n] = t

        dma_xT(0)

        # ---- small/routing inputs first: the LoRA bootstrap + routing DVE
        # prologue only needs ~0.7 MB before the big weights stream in ----
        tp_sb = cons.tile([P, nt * 2], f32)
        nc.sync.dma_start(tp_sb.rearrange("p (n k) -> p n k", k=2),
                          tkp.rearrange("(n p) k -> p n k", p=P))
        ti_sb = cons.tile([P, nt * 4], i32)
        nc.sync.dma_start(ti_sb.rearrange("p (n k) -> p n k", k=4),
                          tki.rearrange("(n p) k -> p n k", p=P))
        wdr = cons.tile([P, NDC * ER], bf16)
        nc.sync.dma_start(wdr.rearrange("p (c e) -> p c e", c=NDC),
                          wd.rearrange("(c p) e -> p c e", p=P))
        wur = cons.tile([ER, D], bf16)
        nc.sync.dma_start(wur[:], wu)
        b1_sb = cons.tile([P, NH], f32)
        nc.sync.dma_start(b1_sb[:], b1.rearrange("(c p) -> p c", p=P))

        # ---- W1: host-prepermuted, 4 chunk-tiles so MM1 h=0 only waits on
        # the first 0.5 MB; need-ordered on the sync DGE ring ----
        # w1 dram layout: w1[p, h*512 + c*128 + q] = W1[c*128 + p, h*128 + q]
        W1c = []
        for hh in range(4):
            t = cons.tile([P, 4 * D], bf16, tag=f"w1c{hh}", name=f"w1c{hh}")
            nc.sync.dma_start(t[:], w1[:, hh * 2048:(hh + 1) * 2048])
            W1c.append(t)
        if 1 < ngroups:
            dma_xT(1)

        # ---- W2 on the Scalar HWDGE ring: streams concurrently with the
        # sync ring so MM2(0) isn't starved behind W1/xT ----
        # w2 dram layout: w2[p, h*512 + d] = W2[h*128 + p, d]
        W2c = []
        for k in range(2):
            t = cons.tile([P, 8 * D], bf16, tag=f"w2c{k}", name=f"w2c{k}")
            nc.scalar.dma_start(t[:], w2[:, k * 4096:(k + 1) * 4096])
            W2c.append(t)
        b2rep = cons.tile([P, D], f32)
        nc.scalar.dma_start(b2rep[:], b2r)

        # ---- routing weights prologue: wts[p, e*nt + n] ----
        idxf = cons.tile([P, nt * 2], f32)
        iv = ti_sb.rearrange("p (n k two) -> p n k two", k=2, two=2)
        nc.vector.tensor_copy(
            idxf.rearrange("p (n k one) -> p n k one", k=2, one=1),
            iv[:, :, :, 0:1])
        wts = cons.tile([P, E * nt], f32)
        for e in range(E):
            eq = cons.tile([P, nt * 2], f32, tag="eq", name=f"eq{e}")
            nc.vector.tensor_scalar(eq[:], idxf[:], float(e), None,
                                    op0=OP.is_equal)
            nc.vector.tensor_tensor(eq[:], eq[:], tp_sb[:], op=OP.mult)
            nc.vector.tensor_reduce(wts[:, e * nt:(e + 1) * nt],
                                    eq.rearrange("p (n k) -> p n k", k=2),
                                    axis=mybir.AxisListType.X, op=OP.add)

        # ---- pipeline stage emitters ----
        gts = {}

        def emit_lora_down(n):
            # pg_all[t, (i, er)] for the whole SG in one PSUM bank, then
            # gelu (ScalarE) and routing scale (DVE) -> g2 bf16
            ng, G, t0 = grp(n)
            pg = ps_lg.tile([P, ng * ER], f32, tag="lg", name=f"pg{n}")
            xTt = xT[n]
            for i in range(ng):
                for c in range(NDC):
                    nc.tensor.matmul(
                        pg[:, i * ER:(i + 1) * ER],
                        xTt[:, c * G + i * P: c * G + (i + 1) * P],
                        wdr[:, c * ER:(c + 1) * ER],
                        start=(c == 0), stop=(c == NDC - 1))
            g_sb = gp.tile([P, ng * ER], f32, tag="g", name=f"g{n}")
            nc.scalar.activation(g_sb[:], pg[:], AF.Gelu)
            g2 = gp.tile([P, ng * ER], bf16, tag="g2", name=f"g2{n}")
            for i in range(ng):
                tt = t0 + i
                for e in range(E):
                    nc.vector.tensor_scalar(
                        g2[:, i * ER + e * R: i * ER + (e + 1) * R],
                        g_sb[:, i * ER + e * R: i * ER + (e + 1) * R],
                        wts[:, e * nt + tt: e * nt + tt + 1], None, op0=OP.mult)
            return g2

        def emit_g_xpose(n, g2):
            ng, G, t0 = grp(n)
            pgt = ps_gt.tile([ER, G], bf16, tag="lgt", name=f"pgt{n}")
            for i in range(ng):
                nc.tensor.matmul(pgt[:, i * P:(i + 1) * P],
                                 g2[:, i * ER:(i + 1) * ER],
                                 ident_b[:], is_transpose=True,
                                 start=True, stop=True)
            gt = gp.tile([ER, G], bf16, tag="gt", name=f"gt{n}")
            nc.vector.tensor_copy(gt[:], pgt[:])
            gts[n] = gt

        # ---- bootstrap: LoRA down-matmuls for SG 0 and 1 (pure PE work);
        # their g-transposes wait on DVE scaling, so they are emitted after
        # MM1(0) inside the loop to keep the in-order PE stream flowing ----
        g2_boot = [emit_lora_down(0)]

        # ---- main loop over supergroups ----
        for n in range(ngroups):
            ng, G, t0 = grp(n)
            if 2 <= n + 2 < ngroups:
                dma_xT(n + 2)

            # MM1 + bias + gelu -> actT [128h, (h_chunk, t)] bf16
            actT = actp.tile([P, NH * G], bf16, tag="actT", name=f"actT{n}")
            xTt = xT[n]
            for h in range(NH):
                ph = ps_h.tile([P, D], f32, tag="ph", name=f"ph{n}_{h}")
                for c in range(NDC):
                    nc.tensor.matmul(
                        ph[:, :G],
                        W1c[h // 4][:, (h % 4) * D + c * P: (h % 4) * D + (c + 1) * P],
                        xTt[:, c * G:(c + 1) * G],
                        start=(c == 0), stop=(c == NDC - 1))
                nc.scalar.activation(actT[:, h * G:(h + 1) * G], ph[:, :G],
                                     AF.Gelu, bias=b1_sb[:, h:h + 1], scale=1.0)

            pending = []
            if n == 0:
                emit_g_xpose(0, g2_boot[0])
                if 1 < ngroups:
                    pending.append((1, emit_lora_down(1)))

            # stage SG n+2 LoRA down (PE work now; its ScalarE/DVE stages
            # drain during MM2 below)
            if n + 2 < ngroups:
                pending.append((n + 2, emit_lora_down(n + 2)))

            # MM2 per token tile: LoRA-up first, then 16 h-chunks
            o4 = outp.tile([P, ng * D], f32, tag="o4", name=f"o4_{n}")
            gt = gts[n]
            for i in range(ng):
                tt = t0 + i
                po = ps_o.tile([P, D], f32, tag="po", name=f"po{tt}")
                nc.tensor.matmul(po[:], gt[:, i * P:(i + 1) * P], wur[:],
                                 start=True, stop=False)
                for h in range(NH):
                    nc.tensor.matmul(
                        po[:],
                        actT[:, h * G + i * P: h * G + (i + 1) * P],
                        W2c[h // 8][:, (h % 8) * D:(h % 8 + 1) * D],
                        start=False, stop=(h == NH - 1))
                nc.vector.tensor_tensor(o4[:, i * D:(i + 1) * D], po[:],
                                        b2rep[:], op=OP.add)
                if (i % 2 == 1 or i == ng - 1) and n < ngroups - 2:
                    lo = (i // 2) * 2
                    cnt = i - lo + 1
                elif n >= ngroups - 2:
                    lo, cnt = i, 1
                else:
                    lo = None
                if lo is not None:
                    nc.sync.dma_start(
                        out[(t0 + lo) * P: (t0 + lo + cnt) * P, :].rearrange(
                            "(i p) d -> p i d", p=P),
                        o4[:, lo * D:(lo + cnt) * D].rearrange(
                            "p (i d) -> p i d", i=cnt))

            del xT[n]
            del gts[n]
            for k, g2p in pending:
                emit_g_xpose(k, g2p)

    nc.compile()
    return nc


def _get_nc():
    key = ("full", TC)
    if key not in _CACHE:
        _CACHE[key] = _build(TC)
    return _CACHE[key]


def _make_in_maps(inputs, tc_tokens=TC, n_cores=N_CORES):
    import ml_dtypes
    bf16 = ml_dtypes.bfloat16

    x = np.asarray(inputs["x"], dtype=np.float32)
    T = x.size // D
    x_flat = x.reshape(T, D).astype(bf16)
    # pre-permute to the SBUF layouts (see _build) for contiguous DMA
    W1 = np.ascontiguousarray(
        np.asarray(inputs["W1"], dtype=np.float32).astype(bf16)
        .reshape(NDC, P, NH, P).transpose(1, 2, 0, 3).reshape(P, NH * D))
    W2 = np.ascontiguousarray(
        np.asarray(inputs["W2"], dtype=np.float32).astype(bf16)
        .reshape(NH, P, D).transpose(1, 0, 2).reshape(P, NH * D))
    b1 = np.ascontiguousarray(inputs["b1"], dtype=np.float32)
    b2 = np.asarray(inputs["b2"], dtype=np.float32)
    b2rep = np.ascontiguousarray(np.broadcast_to(b2[None, :], (P, D)))
    wdn = np.ascontiguousarray(
        np.asarray(inputs["w_down"], dtype=np.float32)
        .transpose(1, 0, 2).reshape(D, ER).astype(bf16))
    wup = np.ascontiguousarray(
        np.asarray(inputs["w_up"], dtype=np.float32).reshape(ER, D).astype(bf16))
    tkp = np.ascontiguousarray(inputs["topk_probs"], dtype=np.float32)
    tki_in = np.asarray(inputs["topk_indices"])
    tki = np.zeros((T, 4), dtype=np.int32)
    tki[:, 0] = tki_in[:, 0]
    tki[:, 2] = tki_in[:, 1]
    ident = np.eye(P, dtype=np.float32).astype(bf16)

    in_maps = []
    for c in range(n_cores):
        sl = slice(c * tc_tokens, (c + 1) * tc_tokens)
        in_maps.append(dict(
            x=np.ascontiguousarray(x_flat[sl].T), w1=W1, w2=W2, b1=b1,
            b2r=b2rep, wd=wdn, wu=wup, tkp=np.ascontiguousarray(tkp[sl]),
            tki=np.ascontiguousarray(tki[sl]), ident=ident))
    return in_maps


def _ensure_ntff_hook():
    """Register the axon NTFF profile hook if the image's antenv lacks it."""
    import sys
    import types
    try:
        from antenv.axon_hooks import get_axon_ntff_profile_hook  # noqa: F401
        return True
    except ImportError:
        pass
    try:
        from trn_agent_boot.trn_boot import _ntff_profile_via_ctypes
        mod = types.ModuleType("antenv.axon_hooks")
        _hook = [None]
        mod.set_axon_ntff_profile_hook = lambda h: _hook.__setitem__(0, h)
        mod.get_axon_ntff_profile_hook = lambda: _hook[0]
        sys.modules["antenv.axon_hooks"] = mod
        import antenv
        antenv.axon_hooks = mod
        mod.set_axon_ntff_profile_hook(
            _ntff_profile_via_ctypes("/opt/axon/libaxon_pjrt.so"))
        return True
    except Exception:
        return False


def kernel(**inputs):
    from concourse.bass_utils import run_bass_kernel_spmd

    nc = _get_nc()
    in_maps = _make_in_maps(inputs)
    trace = bool(int(os.environ.get("KERNEL_TRACE", "0")))
    if trace and not _ensure_ntff_hook():
        trace = False
    res = run_bass_kernel_spmd(nc, in_maps, list(range(N_CORES)), trace=trace)
    if trace:
        _CACHE["last_result"] = res
    out = np.concatenate([res.results[i]["out"] for i in range(N_CORES)], axis=0)
    return out.reshape(np.asarray(inputs["x"]).shape).astype(np.float32)


# revision 11
# speedup vs baseline: 1.0300x; 1.0240x over previous
"""Trainium2 Bass kernel: ConvNeXt MLP + parallel MoE-LoRA (data-parallel over tokens).

Math per token t (D=512, Dh=2048, E=3 experts, r=8, top-k=2):
    base = gelu(x @ W1 + b1) @ W2 + b2
    g_e  = gelu(x @ w_down[e]) * wts[e, t]          (wts from top-k routing)
    out  = base + sum_e g_e @ w_up[e]

Strategy (per NeuronCore, 8 cores data-parallel on the token dim):
  - all matmul operands pre-cast to bf16 on host (x, W1, W2, wd, wu, ident)
    so the device does zero weight casting and x DMA is halved.
  - tokens tiled 128 at a time; supergroups (SG) of 4 tiles (512 tokens).
  - x tile [128t, 512d] PE-transposed into xT [128d, t] (bf16 direct).
  - MM1: hT[h,t] = W1_chunk.T @ xT, fused bias+gelu on ScalarE -> actT bf16.
    W1 SBUF layout is h-major so MM1 h=0 can start after 1/4 of the W1 DMA.
  - MM2: out[t,d] accumulates the rank-24 LoRA-up matmul FIRST, then the
    16 h-chunks, in one PSUM group; b2 added during the drain (DVE) from a
    host-replicated [128,512] bias tile.
  - LoRA chain for SG n+1 (down-matmuls -> gelu -> routing scale -> PE
    transpose -> SBUF) is emitted during SG n so its ScalarE/DVE stages
    hide under MM1/MM2 and the PE never waits on it.
  - routing weights wts[e,t] computed on device in a DVE prologue.
  - batched DMAs: x in per-SG [512 tok], out per 2 tiles, W1 in 4 / W2 in
    2 chunks.
"""

import os
import numpy as np

P = 128
D = 512
DH = 2048
E = 3
R = 8
ER = E * R  # 24
NH = DH // P  # 16
NDC = D // P  # 4
N_CORES = 8
T_FULL = 64 * 28 * 28  # 50176
TC = T_FULL // N_CORES  # 6272
GROUP_TILES = 4

_CACHE = {}


def _build(tc_tokens):
    import concourse.bacc as bacc
    import concourse.tile as tile
    import concourse.mybir as mybir
    from contextlib import ExitStack

    f32 = mybir.dt.float32
    bf16 = mybir.dt.bfloat16
    i32 = mybir.dt.int32
    AF = mybir.ActivationFunctionType
    OP = mybir.AluOpType

    nt = tc_tokens // P  # token tiles (49)
    assert tc_tokens % P == 0
    ngroups = (nt + GROUP_TILES - 1) // GROUP_TILES

    nc = bacc.Bacc("TRN2", target_bir_lowering=False, debug=False,
                   num_devices=N_CORES)

    xt = nc.dram_tensor("x", [D, tc_tokens], bf16, kind="ExternalInput").ap()
    w1 = nc.dram_tensor("w1", [P, NH * D], bf16, kind="ExternalInput").ap()
    w2 = nc.dram_tensor("w2", [P, NH * D], bf16, kind="ExternalInput").ap()
    b1 = nc.dram_tensor("b1", [P, NH], f32, kind="ExternalInput").ap()
    b2r = nc.dram_tensor("b2r", [P, D], f32, kind="ExternalInput").ap()
    wd = nc.dram_tensor("wd", [P, NDC * ER], bf16, kind="ExternalInput").ap()
    wu = nc.dram_tensor("wu", [ER, D], bf16, kind="ExternalInput").ap()
    tkp = nc.dram_tensor("tkp", [tc_tokens, 2], f32, kind="ExternalInput").ap()
    tki = nc.dram_tensor("tki", [tc_tokens, 4], i32, kind="ExternalInput").ap()
    ident_d = nc.dram_tensor("ident", [P, P], bf16, kind="ExternalInput").ap()
    out = nc.dram_tensor("out", [tc_tokens, D], f32, kind="ExternalOutput").ap()

    def grp(n):
        ng = min(GROUP_TILES, nt - n * GROUP_TILES)
        return ng, ng * P, n * GROUP_TILES

    with tile.TileContext(nc) as tc, ExitStack() as ctx:
        cons = ctx.enter_context(tc.tile_pool(name="cons", bufs=1))
        xin = ctx.enter_context(tc.tile_pool(name="xin", bufs=4))
        actp = ctx.enter_context(tc.tile_pool(name="actp", bufs=2))
        outp = ctx.enter_context(tc.tile_pool(name="outp", bufs=2))
        gp = ctx.enter_context(tc.tile_pool(name="gp", bufs=3))
        ps_h = ctx.enter_context(tc.tile_pool(name="ps_h", bufs=3, space="PSUM"))
        ps_o = ctx.enter_context(tc.tile_pool(name="ps_o", bufs=3, space="PSUM"))
        ps_lg = ctx.enter_context(tc.tile_pool(name="ps_lg", bufs=1, space="PSUM"))
        ps_gt = ctx.enter_context(tc.tile_pool(name="ps_gt", bufs=1, space="PSUM"))

        # ---- identity (needed by the very first transposes) ----
        ident_b = cons.tile([P, P], bf16)
        nc.sync.dma_start(ident_b[:], ident_d)

        # ---- x arrives host-transposed: DMA xT [128d(c), (c, t)] per SG ----
        xT = {}

        def dma_xT(n):
            ng, G, t0 = grp(n)
            tok0 = t0 * P
            t = xin.tile([P, NDC * G], bf16, tag="xT", name=f"xT{n}")
            nc.sync.dma_start(
                t.rearrange("p (c t) -> p c t", c=NDC),
                xt.rearrange("(c p) t -> p c t", p=P)[:, :, tok0:tok0 + G])
            xT[n] = t

        dma_xT(0)

        # ---- small/routing inputs first: the LoRA bootstrap + routing DVE
        # prologue only needs ~0.7 MB before the big weights stream in ----
        tp_sb = cons.tile([P, nt * 2], f32)
        nc.sync.dma_start(tp_sb.rearrange("p (n k) -> p n k", k=2),
                          tkp.rearrange("(n p) k -> p n k", p=P))
        ti_sb = cons.tile([P, nt * 4], i32)
        nc.sync.dma_start(ti_sb.rearrange("p (n k) -> p n k", k=4),
                          tki.rearrange("(n p) k -> p n k", p=P))
        wdr = cons.tile([P, NDC * ER], bf16)
        nc.sync.dma_start(wdr[:], wd)
        wur = cons.tile([ER, D], bf16)
        nc.sync.dma_start(wur[:], wu)
        b1_sb = cons.tile([P, NH], f32)
        nc.sync.dma_start(b1_sb[:], b1)

        # ---- W1: host-prepermuted, 4 chunk-tiles so MM1 h=0 only waits on
        # the first 0.5 MB; need-ordered on the sync DGE ring ----
        # w1 dram layout: w1[p, h*512 + c*128 + q] = W1[c*128 + p, h*128 + q]
        W1c = []
        for hh in range(4):
            t = cons.tile([P, 4 * D], bf16, tag=f"w1c{hh}", name=f"w1c{hh}")
            nc.sync.dma_start(t[:], w1[:, hh * 2048:(hh + 1) * 2048])
            W1c.append(t)
        if 1 < ngroups:
            dma_xT(1)

        # ---- W2 on the Scalar HWDGE ring: streams concurrently with the
        # sync ring so MM2(0) isn't starved behind W1/xT ----
        # w2 dram layout: w2[p, h*512 + d] = W2[h*128 + p, d]
        W2c = []
        for k in range(2):
            t = cons.tile([P, 8 * D], bf16, tag=f"w2c{k}", name=f"w2c{k}")
            nc.scalar.dma_start(t[:], w2[:, k * 4096:(k + 1) * 4096])
            W2c.append(t)
        b2rep = cons.tile([P, D], f32)
        nc.scalar.dma_start(b2rep[:], b2r)

        # ---- routing weights prologue: wts[p, e*nt + n] ----
        idxf = cons.tile([P, nt * 2], f32)
        iv = ti_sb.rearrange("p (n k two) -> p n k two", k=2, two=2)
        nc.vector.tensor_copy(
            idxf.rearrange("p (n k one) -> p n k one", k=2, one=1),
            iv[:, :, :, 0:1])
        wts = cons.tile([P, E * nt], f32)
        for e in range(E):
            eq = cons.tile([P, nt * 2], f32, tag="eq", name=f"eq{e}")
            nc.vector.tensor_scalar(eq[:], idxf[:], float(e), None,
                                    op0=OP.is_equal)
            nc.vector.tensor_tensor(eq[:], eq[:], tp_sb[:], op=OP.mult)
            nc.vector.tensor_reduce(wts[:, e * nt:(e + 1) * nt],
                                    eq.rearrange("p (n k) -> p n k", k=2),
                                    axis=mybir.AxisListType.X, op=OP.add)

        # ---- pipeline stage emitters ----
        gts = {}

        def emit_lora_down(n):
            # pg_all[t, (i, er)] for the whole SG in one PSUM bank, then
            # gelu (ScalarE) and routing scale (DVE) -> g2 bf16
            ng, G, t0 = grp(n)
            pg = ps_lg.tile([P, ng * ER], f32, tag="lg", name=f"pg{n}")
            xTt = xT[n]
            for i in range(ng):
                for c in range(NDC):
                    nc.tensor.matmul(
                        pg[:, i * ER:(i + 1) * ER],
                        xTt[:, c * G + i * P: c * G + (i + 1) * P],
                        wdr[:, c * ER:(c + 1) * ER],
                        start=(c == 0), stop=(c == NDC - 1))
            g_sb = gp.tile([P, ng * ER], f32, tag="g", name=f"g{n}")
            nc.scalar.activation(g_sb[:], pg[:], AF.Gelu)
            g2 = gp.tile([P, ng * ER], bf16, tag="g2", name=f"g2{n}")
            for i in range(ng):
                tt = t0 + i
                for e in range(E):
                    nc.vector.tensor_scalar(
                        g2[:, i * ER + e * R: i * ER + (e + 1) * R],
                        g_sb[:, i * ER + e * R: i * ER + (e + 1) * R],
                        wts[:, e * nt + tt: e * nt + tt + 1], None, op0=OP.mult)
            return g2

        def emit_g_xpose(n, g2):
            ng, G, t0 = grp(n)
            pgt = ps_gt.tile([ER, G], bf16, tag="lgt", name=f"pgt{n}")
            for i in range(ng):
                nc.tensor.matmul(pgt[:, i * P:(i + 1) * P],
                                 g2[:, i * ER:(i + 1) * ER],
                                 ident_b[:], is_transpose=True,
                                 start=True, stop=True)
            gt = gp.tile([ER, G], bf16, tag="gt", name=f"gt{n}")
            nc.vector.tensor_copy(gt[:], pgt[:])
            gts[n] = gt

        # ---- bootstrap: LoRA down-matmuls for SG 0 and 1 (pure PE work);
        # their g-transposes wait on DVE scaling, so they are emitted after
        # MM1(0) inside the loop to keep the in-order PE stream flowing ----
        g2_boot = [emit_lora_down(0)]

        # ---- main loop over supergroups ----
        for n in range(ngroups):
            ng, G, t0 = grp(n)
            if 2 <= n + 2 < ngroups:
                dma_xT(n + 2)

            # MM1 + bias + gelu -> actT [128h, (h_chunk, t)] bf16
            actT = actp.tile([P, NH * G], bf16, tag="actT", name=f"actT{n}")
            xTt = xT[n]
            for h in range(NH):
                ph = ps_h.tile([P, D], f32, tag="ph", name=f"ph{n}_{h}")
                for c in range(NDC):
                    nc.tensor.matmul(
                        ph[:, :G],
                        W1c[h // 4][:, (h % 4) * D + c * P: (h % 4) * D + (c + 1) * P],
                        xTt[:, c * G:(c + 1) * G],
                        start=(c == 0), stop=(c == NDC - 1))
                nc.scalar.activation(actT[:, h * G:(h + 1) * G], ph[:, :G],
                                     AF.Gelu, bias=b1_sb[:, h:h + 1], scale=1.0)

            pending = []
            if n == 0:
                emit_g_xpose(0, g2_boot[0])
                if 1 < ngroups:
                    pending.append((1, emit_lora_down(1)))

            # stage SG n+2 LoRA down (PE work now; its ScalarE/DVE stages
            # drain during MM2 below)
            if n + 2 < ngroups:
                pending.append((n + 2, emit_lora_down(n + 2)))

            # MM2 per token tile: LoRA-up first, then 16 h-chunks
            o4 = outp.tile([P, ng * D], f32, tag="o4", name=f"o4_{n}")
            gt = gts[n]
            pos = []
            nup = min(ng, 3)
            for i in range(nup):
                po = ps_o.tile([P, D], f32, tag="po", name=f"po{t0 + i}")
                nc.tensor.matmul(po[:], gt[:, i * P:(i + 1) * P], wur[:],
                                 start=True, stop=False)
                pos.append(po)
            for i in range(ng):
                tt = t0 + i
                if i >= nup:
                    po = ps_o.tile([P, D], f32, tag="po", name=f"po{tt}")
                    nc.tensor.matmul(po[:], gt[:, i * P:(i + 1) * P], wur[:],
                                     start=True, stop=False)
                    pos.append(po)
                po = pos[i]
                for h in range(NH):
                    nc.tensor.matmul(
                        po[:],
                        actT[:, h * G + i * P: h * G + (i + 1) * P],
                        W2c[h // 8][:, (h % 8) * D:(h % 8 + 1) * D],
                        start=False, stop=(h == NH - 1))
                nc.vector.tensor_tensor(o4[:, i * D:(i + 1) * D], po[:],
                                        b2rep[:], op=OP.add)
                if (i % 2 == 1 or i == ng - 1) and n < ngroups - 2:
                    lo = (i // 2) * 2
                    cnt = i - lo + 1
                elif n >= ngroups - 2:
                    lo, cnt = i, 1
                else:
                    lo = None
                if lo is not None:
                    nc.sync.dma_start(
                        out[(t0 + lo) * P: (t0 + lo + cnt) * P, :].rearrange(
                            "(i p) d -> p i d", p=P),
                        o4[:, lo * D:(lo + cnt) * D].rearrange(
                            "p (i d) -> p i d", i=cnt))

            del xT[n]
            del gts[n]
            for k, g2p in pending:
                emit_g_xpose(k, g2p)

    nc.compile()
    return nc


def _get_nc():
    key = ("full", TC)
    if key not in _CACHE:
        _CACHE[key] = _build(TC)
    return _CACHE[key]


def _make_in_maps(inputs, tc_tokens=TC, n_cores=N_CORES):
    import ml_dtypes
    bf16 = ml_dtypes.bfloat16

    x = np.asarray(inputs["x"], dtype=np.float32)
    T = x.size // D
    x_flat = x.reshape(T, D).astype(bf16)
    # pre-permute to the SBUF layouts (see _build) for contiguous DMA
    W1 = np.ascontiguousarray(
        np.asarray(inputs["W1"], dtype=np.float32).astype(bf16)
        .reshape(NDC, P, NH, P).transpose(1, 2, 0, 3).reshape(P, NH * D))
    W2 = np.ascontiguousarray(
        np.asarray(inputs["W2"], dtype=np.float32).astype(bf16)
        .reshape(NH, P, D).transpose(1, 0, 2).reshape(P, NH * D))
    b1 = np.ascontiguousarray(
        np.asarray(inputs["b1"], dtype=np.float32).reshape(NH, P).T)
    b2 = np.asarray(inputs["b2"], dtype=np.float32)
    b2rep = np.ascontiguousarray(np.broadcast_to(b2[None, :], (P, D)))
    wdn = np.ascontiguousarray(
        np.asarray(inputs["w_down"], dtype=np.float32)
        .transpose(1, 0, 2).reshape(NDC, P, ER).transpose(1, 0, 2)
        .reshape(P, NDC * ER).astype(bf16))
    wup = np.ascontiguousarray(
        np.asarray(inputs["w_up"], dtype=np.float32).reshape(ER, D).astype(bf16))
    tkp = np.ascontiguousarray(inputs["topk_probs"], dtype=np.float32)
    tki_in = np.asarray(inputs["topk_indices"])
    tki = np.zeros((T, 4), dtype=np.int32)
    tki[:, 0] = tki_in[:, 0]
    tki[:, 2] = tki_in[:, 1]
    ident = np.eye(P, dtype=np.float32).astype(bf16)

    in_maps = []
    for c in range(n_cores):
        sl = slice(c * tc_tokens, (c + 1) * tc_tokens)
        in_maps.append(dict(
            x=np.ascontiguousarray(x_flat[sl].T), w1=W1, w2=W2, b1=b1,
            b2r=b2rep, wd=wdn, wu=wup, tkp=np.ascontiguousarray(tkp[sl]),
            tki=np.ascontiguousarray(tki[sl]), ident=ident))
    return in_maps


def _ensure_ntff_hook():
    """Register the axon NTFF profile hook if the image's antenv lacks it."""
    import sys
    import types
    try:
        from antenv.axon_hooks import get_axon_ntff_profile_hook  # noqa: F401
        return True
    except ImportError:
        pass
    try:
        from trn_agent_boot.trn_boot import _ntff_profile_via_ctypes
        mod = types.ModuleType("antenv.axon_hooks")
        _hook = [None]
        mod.set_axon_ntff_profile_hook = lambda h: _hook.__setitem__(0, h)
        mod.get_axon_ntff_profile_hook = lambda: _hook[0]
        sys.modules["antenv.axon_hooks"] = mod
        import antenv
        antenv.axon_hooks = mod
        mod.set_axon_ntff_profile_hook(
            _ntff_profile_via_ctypes("/opt/axon/libaxon_pjrt.so"))
        return True
    except Exception:
        return False


def kernel(**inputs):
    from concourse.bass_utils import run_bass_kernel_spmd

    nc = _get_nc()
    in_maps = _make_in_maps(inputs)
    trace = bool(int(os.environ.get("KERNEL_TRACE", "0")))
    if trace and not _ensure_ntff_hook():
        trace = False
    res = run_bass_kernel_spmd(nc, in_maps, list(range(N_CORES)), trace=trace)
    if trace:
        _CACHE["last_result"] = res
    out = np.concatenate([res.results[i]["out"] for i in range(N_CORES)], axis=0)
    return out.reshape(np.asarray(inputs["x"]).shape).astype(np.float32)
